# revision 2
# baseline (speedup 1.0000x reference)
"""Trainium2 Bass kernel for nn_CrossAttention (B=4, C=256, H=W=64).

Sharding: 8 cores = (batch b, query-half h). Each core computes, for its
batch and its half of the query rows i (IH=2048):
  q = Wq x_i + bq        [32, 2048] stored 4x row-replicated as q4 [128, 2048]
  k = Wk x_f             [32, 4096] stored 4x row-replicated as k4 [128, 4096]
                         (bk dropped: constant-in-j shift is softmax-invariant)
  vT = (Wv x_f)^T        [4096, 256] bf16  (bv folded into bc_eff on host)
  S^T[j, i] = k_j . q_i  (transposed: softmax denom + attended matmul need no
                          on-chip transposes; row-replication lets two K=32
                          score matmuls run concurrently in PE row strips)
  E = exp(S^T) bf16      (no max subtraction: |S| <~ 30, exp safe in f32)
  r[i] = sum_j E[j, i]   (DVE-accumulated per group, folded across partitions
                          by one gpsimd C-reduce per query block -- keeps all
                          ones-matmuls off the PE)
  att[c, i] = sum_j vT[j, c] E[j, i] / r[i]
  comb = Wc [x_i; att] + bc_eff ; out[i] = sum_c |comb|  (ones-matmul)
All matmul operands bf16 (fp32r streams at 2 cyc/col and its weight loads
don't hide; bf16 streams at 1). Phase 2a computes att for all query blocks
with the r -> recip -> partition_broadcast -> normalize tail fully off the
PE queue; phase 2b runs every combine back-to-back so the PE never waits
on the softmax-denominator chain.
"""

import numpy as np
import ml_dtypes

import concourse.bass as bass
import concourse.bacc as bacc
import concourse.tile as tile
import concourse.mybir as mybir
from concourse.bass_utils import run_bass_kernel_spmd

B, C, HH, WW = 4, 256, 64, 64
N = HH * WW          # 4096
CQK = 32
IH = N // 2          # 2048 query rows per core
NCORES = 8
NJC = N // 128       # 32 key-dim 128-chunks
NG = NJC // 2        # 16 groups of 2 key-chunks

F32 = mybir.dt.float32
BF16 = mybir.dt.bfloat16
AF = mybir.ActivationFunctionType
ALU = mybir.AluOpType


def build_program(nc, tc):
    # ---- DRAM I/O ------------------------------------------------------
    dram = {}
    for name, shape, dt in [
        ("x1f", [2, 128, N], BF16), ("x2f", [2, 128, N], BF16),
        ("x1i", [2, 128, IH], BF16), ("x2i", [2, 128, IH], BF16),
        ("wqt", [2, 128, 128], BF16), ("wkt", [2, 128, 128], BF16),
        ("wvt", [2, 128, C], BF16),
        ("wctx", [2, 128, C], BF16), ("wcta", [2, 128, C], BF16),
        ("bq", [128, 1], F32), ("bce", [128, 2], F32),
    ]:
        dram[name] = nc.dram_tensor(name, shape, dt, kind="ExternalInput").ap()
    out_d = nc.dram_tensor("out", [2, IH], F32, kind="ExternalOutput").ap()

    import contextlib
    with contextlib.ExitStack() as ctx:
        persist = ctx.enter_context(tc.tile_pool(name="persist", bufs=1))

        wq_sb = persist.tile([128, 2, 128], BF16, tag="wq")
        wk_sb = persist.tile([128, 2, 128], BF16, tag="wk")
        wv_sb = persist.tile([128, 2, C], BF16, tag="wv")
        wcx_sb = persist.tile([128, 2, C], BF16, tag="wcx")
        wca_sb = persist.tile([128, 2, C], BF16, tag="wca")
        bq_sb = persist.tile([128, 1], F32, tag="bq")
        bce_sb = persist.tile([128, 2], F32, tag="bce")
        ones_bf = persist.tile([128, 1], BF16, tag="ones")

        for w, t in [("wqt", wq_sb), ("wkt", wk_sb), ("wvt", wv_sb),
                     ("wctx", wcx_sb), ("wcta", wca_sb)]:
            for kc in range(2):
                nc.sync.dma_start(out=t[:, kc, :], in_=dram[w][kc])
        nc.sync.dma_start(out=bq_sb, in_=dram["bq"])
        nc.sync.dma_start(out=bce_sb, in_=dram["bce"])
        nc.vector.memset(ones_bf, 1.0)

        x1i_sb = [persist.tile([128, IH], BF16, tag=f"x1i{kc}",
                               name=f"x1i{kc}") for kc in range(2)]
        for kc in range(2):
            nc.sync.dma_start(out=x1i_sb[kc], in_=dram["x1i"][kc])

        # projection outputs; k4/vT split in j-halves for earlier consumption
        q4_sb = [persist.tile([128, IH], BF16, tag=f"q{i}", name=f"q{i}")
                 for i in range(2)]
        k4_sb = [[persist.tile([128, N // 2], BF16, tag=f"k{i}{h}",
                               name=f"k{i}{h}") for h in range(2)]
                 for i in range(2)]
        vT_sb = [[persist.tile([128, (NJC // 2) * C], BF16, tag=f"vt{i}{h}",
                               name=f"vt{i}{h}") for h in range(2)]
                 for i in range(2)]
        att_sb = [[persist.tile([128, IH], BF16, tag=f"att{br}{c2}",
                                name=f"att{br}{c2}") for c2 in range(2)]
                  for br in range(2)]

        # ---- phase 1: projections -------------------------------------
        with tc.tile_pool(name="proj_sb", bufs=2) as proj_sb, \
             tc.tile_pool(name="ps_kq", bufs=3, space="PSUM") as ps_kq, \
             tc.tile_pool(name="ps_vt", bufs=2, space="PSUM") as ps_vt:

            # q4 projections (from islice inputs; bq folded via ACT bias)
            for xi in range(2):
                if xi == 0:
                    xi_sb = x1i_sb
                else:
                    xi_sb = [proj_sb.tile([128, IH], BF16, tag="x2i",
                                          name="x2i") for _ in range(2)]
                    for kc in range(2):
                        nc.sync.dma_start(out=xi_sb[kc], in_=dram["x2i"][kc])
                for ib in range(4):
                    sl = bass.ts(ib, 512)
                    qp = ps_kq.tile([128, 512], F32, tag="kq", name="qp")
                    for kc in range(2):
                        nc.tensor.matmul(qp, wq_sb[:, kc, :], xi_sb[kc][:, sl],
                                         start=(kc == 0), stop=(kc == 1))
                    nc.scalar.activation(q4_sb[xi][:, sl], qp, AF.Identity,
                                         bias=bq_sb)

            # k4 and vT projections, x2 first (branch 0 needs vT2)
            for xi, xf_name in [(1, "x2f"), (0, "x1f")]:
                for jh in range(2):
                    xf_t = proj_sb.tile([128, 2, IH], BF16, tag="xf",
                                        name="xf")
                    for kc in range(2):
                        nc.sync.dma_start(
                            out=xf_t[:, kc, :],
                            in_=dram[xf_name][kc][:, jh * IH:(jh + 1) * IH])
                    for jb in range(4):
                        sl = bass.ts(jb, 512)
                        kp = ps_kq.tile([128, 512], F32, tag="kq", name="kp")
                        for kc in range(2):
                            nc.tensor.matmul(kp, wk_sb[:, kc, :],
                                             xf_t[:, kc, sl],
                                             start=(kc == 0), stop=(kc == 1))
                        nc.scalar.activation(k4_sb[xi][jh][:, sl], kp, AF.Copy)
                    for g in range(4):
                        vtp = ps_vt.tile([128, 4, C], F32, tag="vt",
                                         name="vtp")
                        for s in range(4):
                            jsub = g * 4 + s
                            for kc in range(2):
                                nc.tensor.matmul(
                                    vtp[:, s, :],
                                    xf_t[:, kc, bass.ts(jsub, 128)],
                                    wv_sb[:, kc, :],
                                    start=(kc == 0), stop=(kc == 1))
                        nc.vector.tensor_copy(
                            vT_sb[xi][jh][:, bass.ds(g * 4 * C, 4 * C)],
                            vtp.rearrange("p a c -> p (a c)"))

        # ---- phase 2a: attention (att_sb <- normalized attended) -------
        with tc.tile_pool(name="attn_sb", bufs=1) as attn_sb, \
             tc.tile_pool(name="ps_att", bufs=1, space="PSUM") as ps_att, \
             tc.tile_pool(name="ps_st", bufs=1, space="PSUM") as ps_st:

            for br in range(2):
                q4, k4, vT = q4_sb[br], k4_sb[br], vT_sb[1 - br]
                for ib in range(4):
                    isl = bass.ts(ib, 512)
                    attp = [ps_att.tile([128, 512], F32, tag="attp",
                                        bufs=4, name=f"attp{c2}")
                            for c2 in range(2)]
                    racc = None
                    for g in range(NG):
                        jcs = (2 * g, 2 * g + 1)
                        jh = g // (NG // 2)
                        jloc = [jc - jh * (NJC // 2) for jc in jcs]
                        stp = ps_st.tile([128, 2, 512], F32, tag="stp",
                                         bufs=2, name="stp")
                        # score matmuls: 2 row strips run concurrently
                        for t in range(2):
                            nc.tensor.matmul(
                                stp[:, t, :],
                                k4[jh][32 * t:32 * (t + 1),
                                       bass.ts(jloc[t], 128)],
                                q4[32 * t:32 * (t + 1), isl],
                                start=True, stop=True,
                                tile_position=(32 * t, 0))
                        est = attn_sb.tile([128, 2, 512], BF16,
                                           tag="est", bufs=6, name="est")
                        nc.scalar.activation(
                            est.rearrange("p a n -> p (a n)"),
                            stp.rearrange("p a n -> p (a n)"), AF.Exp)
                        # attended
                        for t in range(2):
                            for c2 in range(2):
                                nc.tensor.matmul(
                                    attp[c2],
                                    vT[jh][:, bass.ds(jloc[t] * C
                                                      + c2 * 128, 128)],
                                    est[:, t, :],
                                    start=(g == 0 and t == 0),
                                    stop=(g == NG - 1 and t == 1))
                        # r accumulation fully off the PE: pair-sum the two
                        # strips (bf16, 2x DVE rate), accumulate in f32
                        rtmp = attn_sb.tile([128, 512], BF16, tag="rtmp",
                                            bufs=2, name="rtmp")
                        nc.vector.tensor_tensor(rtmp, est[:, 0, :],
                                                est[:, 1, :], ALU.add)
                        racc_new = attn_sb.tile([128, 512], F32, tag="racc",
                                                bufs=3, name="racc")
                        if racc is None:
                            nc.vector.tensor_copy(racc_new, rtmp)
                        else:
                            nc.vector.tensor_tensor(racc_new, racc, rtmp,
                                                    ALU.add)
                        racc = racc_new
                    # fold r across partitions (gpsimd), recip, broadcast,
                    # normalize -- none of it on the tensor engine
                    rfold = attn_sb.tile([1, 512], F32, tag="rfold", bufs=2,
                                         name="rfold")
                    nc.gpsimd.tensor_reduce(rfold, racc,
                                            mybir.AxisListType.C, ALU.add)
                    rr = attn_sb.tile([1, 512], F32, tag="rr", bufs=2,
                                      name="rr")
                    nc.vector.reciprocal(rr, rfold)
                    rrb = attn_sb.tile([128, 512], F32, tag="rrb", bufs=2,
                                       name="rrb")
                    nc.gpsimd.partition_broadcast(rrb, rr)
                    for c2 in range(2):
                        nc.vector.tensor_mul(att_sb[br][c2][:, isl],
                                             attp[c2], rrb)

        # ---- phase 2b: combines, back-to-back on the PE ----------------
        with tc.tile_pool(name="cmb_sb", bufs=1) as cmb_sb, \
             tc.tile_pool(name="ps_cmb", bufs=1, space="PSUM") as ps_cmb:

            for br in range(2):
                for ib in range(4):
                    isl = bass.ts(ib, 512)
                    absb = []
                    for c2 in range(2):
                        cp = ps_cmb.tile([128, 512], F32, tag="cp",
                                         bufs=3, name="cp")
                        for kc in range(2):
                            nc.tensor.matmul(
                                cp, wcx_sb[:, kc, bass.ts(c2, 128)],
                                x1i_sb[kc][:, isl],
                                start=(kc == 0), stop=False)
                        for kc in range(2):
                            nc.tensor.matmul(
                                cp, wca_sb[:, kc, bass.ts(c2, 128)],
                                att_sb[br][kc][:, isl],
                                start=False, stop=(kc == 1))
                        ab = cmb_sb.tile([128, 512], BF16, tag="absb",
                                         bufs=4, name="absb")
                        nc.scalar.activation(ab, cp, AF.Abs,
                                             bias=bce_sb[:, c2:c2 + 1])
                        absb.append(ab)
                    outp = ps_cmb.tile([1, 512], F32, tag="outp", bufs=2,
                                       name="outp")
                    for c2 in range(2):
                        nc.tensor.matmul(outp, ones_bf, absb[c2],
                                         start=(c2 == 0), stop=(c2 == 1))
                    osb = cmb_sb.tile([1, 512], F32, tag="osb", bufs=2,
                                      name="osb")
                    nc.vector.tensor_copy(osb, outp)
                    nc.sync.dma_start(out=out_d[br:br + 1, isl], in_=osb)


_NC_CACHE = {}


def _get_nc():
    if "nc" not in _NC_CACHE:
        nc = bacc.Bacc("TRN2", debug=False, enable_asserts=False,
                       target_bir_lowering=False, enable_partition_id=False)
        with tile.TileContext(nc) as tc:
            build_program(nc, tc)
        nc.compile()
        _NC_CACHE["nc"] = nc
    return _NC_CACHE["nc"]


def host_inputs(x1, x2, Wq, bq, Wk, bk, Wv, bv, Wc, bc):
    """Build the 8 per-core input maps (host-side sharding/layout only)."""
    f = np.float32
    bf = ml_dtypes.bfloat16
    x1 = np.asarray(x1, f); x2 = np.asarray(x2, f)
    Wq = np.asarray(Wq, f); bq = np.asarray(bq, f)
    Wk = np.asarray(Wk, f)
    Wv = np.asarray(Wv, f); bv = np.asarray(bv, f)
    Wc = np.asarray(Wc, f); bc = np.asarray(bc, f)

    # 4x row-replicated q/k projection weights -> q4/k4 [128, n] layouts
    Wq4 = np.tile(Wq, (4, 1))            # [128, 256]
    Wk4 = np.tile(Wk, (4, 1))
    wqt = np.ascontiguousarray(Wq4.T.reshape(2, 128, 128)).astype(bf)
    wkt = np.ascontiguousarray(Wk4.T.reshape(2, 128, 128)).astype(bf)
    bq4 = np.tile(bq, 4).reshape(128, 1).copy()
    wvt = np.ascontiguousarray(Wv.T.reshape(2, 128, C)).astype(bf)
    WcT = np.ascontiguousarray(Wc.T)     # [512, 256]
    wctx = WcT[:C].reshape(2, 128, C).astype(bf)
    wcta = WcT[C:].reshape(2, 128, C).astype(bf)
    bce = (bc + Wc[:, C:] @ bv).reshape(2, 128).T.copy()   # [128, 2]

    in_maps = []
    for core in range(NCORES):
        b, h = divmod(core, 2)
        x1f = x1[b].reshape(C, N).reshape(2, 128, N)
        x2f = x2[b].reshape(C, N).reshape(2, 128, N)
        in_maps.append({
            "x1f": np.ascontiguousarray(x1f).astype(bf),
            "x2f": np.ascontiguousarray(x2f).astype(bf),
            "x1i": np.ascontiguousarray(
                x1f[:, :, h * IH:(h + 1) * IH]).astype(bf),
            "x2i": np.ascontiguousarray(
                x2f[:, :, h * IH:(h + 1) * IH]).astype(bf),
            "wqt": wqt, "wkt": wkt, "wvt": wvt,
            "wctx": wctx, "wcta": wcta,
            "bq": bq4, "bce": bce,
        })
    return in_maps


def assemble(results):
    """results: list of 8 dicts with 'out' [2, IH] -> (out1, out2) full."""
    outs = []
    for row in range(2):
        full = np.empty((B, 1, HH, WW), np.float32)
        for b in range(B):
            half0 = results[2 * b]["out"][row]
            half1 = results[2 * b + 1]["out"][row]
            full[b, 0] = np.concatenate([half0, half1]).reshape(HH, WW)
        outs.append(full)
    return outs[0], outs[1]


def kernel(x1, x2, Wq, bq, Wk, bk, Wv, bv, Wc, bc):
    in_maps = host_inputs(x1, x2, Wq, bq, Wk, bk, Wv, bv, Wc, bc)
    nc = _get_nc()
    res = run_bass_kernel_spmd(nc, in_maps, core_ids=list(range(NCORES)))
    return assemble(res.results)


# revision 6
# speedup vs baseline: 2.5967x; 2.5967x over previous
"""Trainium2 Bass kernel for nn_CrossAttention (B=4, C=256, H=W=64).

Sharding: 8 cores = (batch b, query-half h). Each core computes, for its
batch and its half of the query rows i (IH=2048):
  q = Wq x_i + bq        [32, 2048] stored 4x row-replicated as q4 [128, 2048]
  k = Wk x_f             [32, 4096] stored 4x row-replicated as k4 [128, 4096]
                         (bk dropped: constant-in-j shift is softmax-invariant)
  vT = (Wv x_f)^T        [4096, 256] bf16  (bv folded into bc_eff on host)
  S^T[j, i] = k_j . q_i  (transposed: softmax denom + attended matmul need no
                          on-chip transposes; row-replication lets two K=32
                          score matmuls run concurrently in PE row strips)
  E = exp(S^T) bf16      (no max subtraction: |S| <~ 30, exp safe in f32)
  r[i] = sum_j E[j, i]   (DVE-accumulated per group, folded across partitions
                          by one gpsimd C-reduce per query block -- keeps all
                          ones-matmuls off the PE)
  att[c, i] = sum_j vT[j, c] E[j, i] / r[i]
  comb = Wc [x_i; att] + bc_eff ; out[i] = sum_c |comb|  (ones-matmul)
All matmul operands bf16 (fp32r streams at 2 cyc/col and its weight loads
don't hide; bf16 streams at 1). Phase 2a computes att for all query blocks
with the r -> recip -> partition_broadcast -> normalize tail fully off the
PE queue; phase 2b runs every combine back-to-back so the PE never waits
on the softmax-denominator chain.
"""

import numpy as np
import ml_dtypes

import concourse.bass as bass
import concourse.bacc as bacc
import concourse.tile as tile
import concourse.mybir as mybir
from concourse.bass_utils import run_bass_kernel_spmd

B, C, HH, WW = 4, 256, 64, 64
N = HH * WW          # 4096
CQK = 32
IH = N // 2          # 2048 query rows per core
NCORES = 8
NJC = N // 128       # 32 key-dim 128-chunks
NG = NJC // 2        # 16 groups of 2 key-chunks

F32 = mybir.dt.float32
BF16 = mybir.dt.bfloat16
AF = mybir.ActivationFunctionType
ALU = mybir.AluOpType


def build_program(nc, tc):
    # ---- DRAM I/O ------------------------------------------------------
    dram = {}
    for name, shape, dt in [
        ("x1f", [2, 128, N], BF16), ("x2f", [2, 128, N], BF16),
        ("x1i", [2, 128, IH], BF16), ("x2i", [2, 128, IH], BF16),
        ("wqt", [2, 128, 128], BF16), ("wkt", [2, 128, 128], BF16),
        ("wvt", [2, 128, C], BF16),
        ("wctx", [2, 128, C], BF16), ("wcta", [2, 128, C], BF16),
        ("bq", [128, 1], F32), ("bce", [128, 2], F32),
    ]:
        dram[name] = nc.dram_tensor(name, shape, dt, kind="ExternalInput").ap()
    out_d = nc.dram_tensor("out", [2, IH], F32, kind="ExternalOutput").ap()

    import contextlib
    with contextlib.ExitStack() as ctx:
        persist = ctx.enter_context(tc.tile_pool(name="persist", bufs=1))

        wq_sb = persist.tile([128, 2, 128], BF16, tag="wq")
        wk_sb = persist.tile([128, 2, 128], BF16, tag="wk")
        wv_sb = persist.tile([128, 2, C], BF16, tag="wv")
        wcx_sb = persist.tile([128, 2, C], BF16, tag="wcx")
        wca_sb = persist.tile([128, 2, C], BF16, tag="wca")
        bq_sb = persist.tile([128, 1], F32, tag="bq")
        bce_sb = persist.tile([128, 2], F32, tag="bce")
        ones_bf = persist.tile([128, 1], BF16, tag="ones")

        for w, t in [("wqt", wq_sb), ("wkt", wk_sb), ("wvt", wv_sb),
                     ("wctx", wcx_sb), ("wcta", wca_sb)]:
            for kc in range(2):
                nc.sync.dma_start(out=t[:, kc, :], in_=dram[w][kc])
        nc.sync.dma_start(out=bq_sb, in_=dram["bq"])
        nc.sync.dma_start(out=bce_sb, in_=dram["bce"])
        nc.vector.memset(ones_bf, 1.0)

        x1i_sb = [persist.tile([128, IH], BF16, tag=f"x1i{kc}",
                               name=f"x1i{kc}") for kc in range(2)]
        for kc in range(2):
            nc.sync.dma_start(out=x1i_sb[kc], in_=dram["x1i"][kc])

        # projection outputs; k4/vT split in j-halves for earlier consumption
        q4_sb = [persist.tile([128, IH], BF16, tag=f"q{i}", name=f"q{i}")
                 for i in range(2)]
        k4_sb = [[persist.tile([128, N // 2], BF16, tag=f"k{i}{h}",
                               name=f"k{i}{h}") for h in range(2)]
                 for i in range(2)]
        vT_sb = [[persist.tile([128, (NJC // 2) * C], BF16, tag=f"vt{i}{h}",
                               name=f"vt{i}{h}") for h in range(2)]
                 for i in range(2)]
        att_sb = [[persist.tile([128, IH], BF16, tag=f"att{br}{c2}",
                                name=f"att{br}{c2}") for c2 in range(2)]
                  for br in range(2)]

        # ---- phase 1: projections -------------------------------------
        with tc.tile_pool(name="proj_sb", bufs=2) as proj_sb, \
             tc.tile_pool(name="ps_kq", bufs=3, space="PSUM") as ps_kq, \
             tc.tile_pool(name="ps_vt", bufs=2, space="PSUM") as ps_vt:

            # q4 projections (from islice inputs; bq folded via ACT bias)
            for xi in range(2):
                if xi == 0:
                    xi_sb = x1i_sb
                else:
                    xi_sb = [proj_sb.tile([128, IH], BF16, tag="x2i",
                                          name="x2i") for _ in range(2)]
                    for kc in range(2):
                        nc.sync.dma_start(out=xi_sb[kc], in_=dram["x2i"][kc])
                for ib in range(4):
                    sl = bass.ts(ib, 512)
                    qp = ps_kq.tile([128, 512], F32, tag="kq", name="qp")
                    for kc in range(2):
                        nc.tensor.matmul(qp, wq_sb[:, kc, :], xi_sb[kc][:, sl],
                                         start=(kc == 0), stop=(kc == 1))
                    nc.scalar.activation(q4_sb[xi][:, sl], qp, AF.Identity,
                                         bias=bq_sb)

            # k4 and vT projections, x2 first (branch 0 needs vT2)
            for xi, xf_name in [(1, "x2f"), (0, "x1f")]:
                for jh in range(2):
                    xf_t = proj_sb.tile([128, 2, IH], BF16, tag="xf",
                                        name="xf")
                    for kc in range(2):
                        nc.sync.dma_start(
                            out=xf_t[:, kc, :],
                            in_=dram[xf_name][kc][:, jh * IH:(jh + 1) * IH])
                    for jb in range(4):
                        sl = bass.ts(jb, 512)
                        kp = ps_kq.tile([128, 512], F32, tag="kq", name="kp")
                        for kc in range(2):
                            nc.tensor.matmul(kp, wk_sb[:, kc, :],
                                             xf_t[:, kc, sl],
                                             start=(kc == 0), stop=(kc == 1))
                        nc.scalar.activation(k4_sb[xi][jh][:, sl], kp, AF.Copy)
                    for g in range(4):
                        vtp = ps_vt.tile([128, 4, C], F32, tag="vt",
                                         name="vtp")
                        for s in range(4):
                            jsub = g * 4 + s
                            for kc in range(2):
                                nc.tensor.matmul(
                                    vtp[:, s, :],
                                    xf_t[:, kc, bass.ts(jsub, 128)],
                                    wv_sb[:, kc, :],
                                    start=(kc == 0), stop=(kc == 1))
                        nc.vector.tensor_copy(
                            vT_sb[xi][jh][:, bass.ds(g * 4 * C, 4 * C)],
                            vtp.rearrange("p a c -> p (a c)"))

        # ---- phase 2a: attention (att_sb <- normalized attended) -------
        with tc.tile_pool(name="attn_sb", bufs=1) as attn_sb, \
             tc.tile_pool(name="ps_att", bufs=1, space="PSUM") as ps_att, \
             tc.tile_pool(name="ps_st", bufs=1, space="PSUM") as ps_st:

            for br in range(2):
                q4, k4, vT = q4_sb[br], k4_sb[br], vT_sb[1 - br]
                for ib in range(4):
                    isl = bass.ts(ib, 512)
                    attp = [ps_att.tile([128, 512], F32, tag="attp",
                                        bufs=4, name=f"attp{c2}")
                            for c2 in range(2)]
                    racc = None
                    for g in range(NG):
                        jcs = (2 * g, 2 * g + 1)
                        jh = g // (NG // 2)
                        jloc = [jc - jh * (NJC // 2) for jc in jcs]
                        stp = ps_st.tile([128, 2, 512], F32, tag="stp",
                                         bufs=2, name="stp")
                        # score matmuls: 2 row strips run concurrently
                        for t in range(2):
                            nc.tensor.matmul(
                                stp[:, t, :],
                                k4[jh][32 * t:32 * (t + 1),
                                       bass.ts(jloc[t], 128)],
                                q4[32 * t:32 * (t + 1), isl],
                                start=True, stop=True,
                                tile_position=(32 * t, 0))
                        est = attn_sb.tile([128, 2, 512], BF16,
                                           tag="est", bufs=6, name="est")
                        nc.scalar.activation(
                            est.rearrange("p a n -> p (a n)"),
                            stp.rearrange("p a n -> p (a n)"), AF.Exp)
                        # attended
                        for t in range(2):
                            for c2 in range(2):
                                nc.tensor.matmul(
                                    attp[c2],
                                    vT[jh][:, bass.ds(jloc[t] * C
                                                      + c2 * 128, 128)],
                                    est[:, t, :],
                                    start=(g == 0 and t == 0),
                                    stop=(g == NG - 1 and t == 1))
                        # r accumulation fully off the PE: pair-sum the two
                        # strips (bf16, 2x DVE rate), accumulate in f32
                        rtmp = attn_sb.tile([128, 512], BF16, tag="rtmp",
                                            bufs=2, name="rtmp")
                        nc.vector.tensor_tensor(rtmp, est[:, 0, :],
                                                est[:, 1, :], ALU.add)
                        racc_new = attn_sb.tile([128, 512], F32, tag="racc",
                                                bufs=3, name="racc")
                        if racc is None:
                            nc.vector.tensor_copy(racc_new, rtmp)
                        else:
                            nc.vector.tensor_tensor(racc_new, racc, rtmp,
                                                    ALU.add)
                        racc = racc_new
                    # fold r across partitions via in-place DMA-accumulate
                    # halvings (DVE tensor_tensor requires equal base
                    # partitions on HW, so the tree runs on the DMA engines
                    # -- entirely off the PE/DVE)
                    for p in (64, 32, 16, 8, 4, 2, 1):
                        nc.gpsimd.dma_start(out=racc[0:p, :],
                                            in_=racc[p:2 * p, :],
                                            accum_op=ALU.add)
                    rr = attn_sb.tile([1, 512], F32, tag="rr", bufs=2,
                                      name="rr")
                    nc.vector.reciprocal(rr, racc[0:1, :])
                    rrb = attn_sb.tile([128, 512], F32, tag="rrb", bufs=2,
                                       name="rrb")
                    nc.gpsimd.partition_broadcast(rrb, rr)
                    for c2 in range(2):
                        nc.vector.tensor_mul(att_sb[br][c2][:, isl],
                                             attp[c2], rrb)

        # ---- phase 2b: combines, back-to-back on the PE ----------------
        with tc.tile_pool(name="cmb_sb", bufs=1) as cmb_sb, \
             tc.tile_pool(name="ps_cmb", bufs=1, space="PSUM") as ps_cmb:

            for br in range(2):
                for ib in range(4):
                    isl = bass.ts(ib, 512)
                    absb = []
                    for c2 in range(2):
                        cp = ps_cmb.tile([128, 512], F32, tag="cp",
                                         bufs=3, name="cp")
                        for kc in range(2):
                            nc.tensor.matmul(
                                cp, wcx_sb[:, kc, bass.ts(c2, 128)],
                                x1i_sb[kc][:, isl],
                                start=(kc == 0), stop=False)
                        for kc in range(2):
                            nc.tensor.matmul(
                                cp, wca_sb[:, kc, bass.ts(c2, 128)],
                                att_sb[br][kc][:, isl],
                                start=False, stop=(kc == 1))
                        ab = cmb_sb.tile([128, 512], BF16, tag="absb",
                                         bufs=4, name="absb")
                        nc.scalar.activation(ab, cp, AF.Abs,
                                             bias=bce_sb[:, c2:c2 + 1])
                        absb.append(ab)
                    outp = ps_cmb.tile([1, 512], F32, tag="outp", bufs=2,
                                       name="outp")
                    for c2 in range(2):
                        nc.tensor.matmul(outp, ones_bf, absb[c2],
                                         start=(c2 == 0), stop=(c2 == 1))
                    osb = cmb_sb.tile([1, 512], F32, tag="osb", bufs=2,
                                      name="osb")
                    nc.vector.tensor_copy(osb, outp)
                    nc.sync.dma_start(out=out_d[br:br + 1, isl], in_=osb)


_NC_CACHE = {}


def _get_nc():
    if "nc" not in _NC_CACHE:
        nc = bacc.Bacc("TRN2", debug=False, enable_asserts=False,
                       target_bir_lowering=False, enable_partition_id=False)
        with tile.TileContext(nc) as tc:
            build_program(nc, tc)
        nc.compile()
        _NC_CACHE["nc"] = nc
    return _NC_CACHE["nc"]


def host_inputs(x1, x2, Wq, bq, Wk, bk, Wv, bv, Wc, bc):
    """Build the 8 per-core input maps (host-side sharding/layout only)."""
    f = np.float32
    bf = ml_dtypes.bfloat16
    x1 = np.asarray(x1, f); x2 = np.asarray(x2, f)
    Wq = np.asarray(Wq, f); bq = np.asarray(bq, f)
    Wk = np.asarray(Wk, f)
    Wv = np.asarray(Wv, f); bv = np.asarray(bv, f)
    Wc = np.asarray(Wc, f); bc = np.asarray(bc, f)

    # 4x row-replicated q/k projection weights -> q4/k4 [128, n] layouts
    Wq4 = np.tile(Wq, (4, 1))            # [128, 256]
    Wk4 = np.tile(Wk, (4, 1))
    wqt = np.ascontiguousarray(Wq4.T.reshape(2, 128, 128)).astype(bf)
    wkt = np.ascontiguousarray(Wk4.T.reshape(2, 128, 128)).astype(bf)
    bq4 = np.tile(bq, 4).reshape(128, 1).copy()
    wvt = np.ascontiguousarray(Wv.T.reshape(2, 128, C)).astype(bf)
    WcT = np.ascontiguousarray(Wc.T)     # [512, 256]
    wctx = WcT[:C].reshape(2, 128, C).astype(bf)
    wcta = WcT[C:].reshape(2, 128, C).astype(bf)
    bce = (bc + Wc[:, C:] @ bv).reshape(2, 128).T.copy()   # [128, 2]

    in_maps = []
    for core in range(NCORES):
        b, h = divmod(core, 2)
        x1f = x1[b].reshape(C, N).reshape(2, 128, N)
        x2f = x2[b].reshape(C, N).reshape(2, 128, N)
        in_maps.append({
            "x1f": np.ascontiguousarray(x1f).astype(bf),
            "x2f": np.ascontiguousarray(x2f).astype(bf),
            "x1i": np.ascontiguousarray(
                x1f[:, :, h * IH:(h + 1) * IH]).astype(bf),
            "x2i": np.ascontiguousarray(
                x2f[:, :, h * IH:(h + 1) * IH]).astype(bf),
            "wqt": wqt, "wkt": wkt, "wvt": wvt,
            "wctx": wctx, "wcta": wcta,
            "bq": bq4, "bce": bce,
        })
    return in_maps


def assemble(results):
    """results: list of 8 dicts with 'out' [2, IH] -> (out1, out2) full."""
    outs = []
    for row in range(2):
        full = np.empty((B, 1, HH, WW), np.float32)
        for b in range(B):
            half0 = results[2 * b]["out"][row]
            half1 = results[2 * b + 1]["out"][row]
            full[b, 0] = np.concatenate([half0, half1]).reshape(HH, WW)
        outs.append(full)
    return outs[0], outs[1]


def kernel(x1, x2, Wq, bq, Wk, bk, Wv, bv, Wc, bc):
    in_maps = host_inputs(x1, x2, Wq, bq, Wk, bk, Wv, bv, Wc, bc)
    nc = _get_nc()
    res = run_bass_kernel_spmd(nc, in_maps, core_ids=list(range(NCORES)))
    return assemble(res.results)
